# revision 1
# baseline (speedup 1.0000x reference)
"""NATTEN-style dilated neighborhood-attention transformer on 8 trn2 cores.

Design:
- Dilation-3 NA factorizes into 9 independent (row-class s, col-class r)
  blocks; in class space each block is a dense 16 x nc map with a k=13, d=1
  neighborhood.  Sharding: core c owns class-block c (c=0..7); block 8's
  queries are split 42/core (2 class-rows each).  All residual-stream ops
  (LN, QKV, proj, MLP) are token-local; only block-8 attention needs an
  AllGather of each rank's 42-token k/vT slice (~86KB/rank) per layer.
- Attention math per block: masked-dense scores S^T[k, q] (keys on
  partitions), exp-arg = (S + T2a) * T1 with host-precomputed tables
  T1 = ker*M + (1-M)  (per layer),
  T2a = rpb_rel*M - 30*(1-M)  (per layer, head; rpb is pure-relative in
  class space: rpb[ku-u+12, kc-qc+12]).
  exp on ACT; softmax sum via ones-matmul over partitions; AV with
  token-major vT (computed by a dedicated transposed matmul); divide via
  4-head block-broadcast matmul + DVE multiply.
- Matmuls use float32r (fp32 data, 1 cycle/row at free-dim>=256).
"""
import numpy as np
import sys, os
sys.path.insert(0, "/opt/trn_rl_repo")

import concourse.bass as bass
import concourse.tile as tile
from concourse import mybir, bacc
from concourse.bass_utils import run_bass_kernel_spmd

F32 = mybir.dt.float32
F32R = mybir.dt.float32r
BF16 = mybir.dt.bfloat16
AFT = mybir.ActivationFunctionType

KSZ, DIL, SIGMA, SC, EPS = 13, 3, 9.0, 0.1, 1e-5
DEPTH, DIM, HEADS = 6, 256, 8
HD = DIM // HEADS
H, W = 48, 64
NU = H // DIL                      # 16 class rows
NCOLS = [22, 21, 21]               # class cols for r=0,1,2
NB = KSZ // 2                      # 6
NTOK = 394                         # 352 (block, padded) + 42 (block-8 slice)
NQA, NQB = 352, 42
NKEY = 384                         # 3 k-tiles of 128
N_CORES = 8
NEG = -30.0

_cache = {}


def _win_start(L):
    return np.clip(np.arange(L) - NB, 0, L - KSZ)


def _host_prep(inputs):
    """Precompute per-core input tensors (numpy)."""
    x = np.asarray(inputs["x"], np.float32).reshape(DIM, H * W)
    rpb = np.asarray(inputs["rpb"], np.float32)
    lr_m = np.asarray(inputs["lr_m"], np.float32)
    crd = np.arange(KSZ, dtype=np.float32)
    g = np.exp(-((crd[None, :] - KSZ // 2) ** 2 + (crd[:, None] - KSZ // 2) ** 2)
               / (2.0 * SIGMA ** 2))
    kers = [g + lr_m[l] / SC for l in range(DEPTH)]          # (13,13) per layer

    # block token coords: block b=3s+r -> flat hw indices, (u,qc) row-major
    blk_tok = []
    for s in range(3):
        for r in range(3):
            nc_ = NCOLS[r]
            u, qc = np.meshgrid(np.arange(NU), np.arange(nc_), indexing="ij")
            blk_tok.append(((3 * u + s) * W + (3 * qc + r)).reshape(-1))

    # per-block T1/T2a in class space, [nkeys=16*nc, nq=16*nc]
    def block_tables(nc_):
        key = ("bt", nc_)
        if key not in _cache:
            su, sw = _win_start(NU), _win_start(nc_)
            KU, KC, U, QC = np.meshgrid(np.arange(NU), np.arange(nc_),
                                        np.arange(NU), np.arange(nc_), indexing="ij")
            m = ((KU >= su[U]) & (KU <= su[U] + KSZ - 1)
                 & (KC >= sw[QC]) & (KC <= sw[QC] + KSZ - 1))
            i = np.where(m, KU - su[U], 0)
            j = np.where(m, KC - sw[QC], 0)
            rr = np.where(m, KU - U + KSZ - 1, 0)
            cc = np.where(m, KC - QC + KSZ - 1, 0)
            _cache[key] = (m.reshape(NU * nc_, NU * nc_), i.reshape(NU * nc_, -1),
                           j.reshape(NU * nc_, -1), rr.reshape(NU * nc_, -1),
                           cc.reshape(NU * nc_, -1))
        return _cache[key]

    # assemble per-core tables [DEPTH, (HEADS), NKEY, NTOK]
    T1 = np.zeros((N_CORES, DEPTH, NKEY, NTOK), np.float32)
    T2 = np.full((N_CORES, DEPTH, HEADS, NKEY, NTOK), 0.0, np.float32)
    x_own = np.zeros((N_CORES, DIM, NTOK), np.float32)
    tokmap = []   # per core: global flat-hw index per col (or -1 pad)
    for c in range(N_CORES):
        ncA = NCOLS[c % 3]
        ntA = NU * ncA
        mA, iA, jA, rA, cA = block_tables(ncA)
        mB, iB, jB, rB, cB = block_tables(21)
        tm = np.full(NTOK, -1, np.int64)
        tm[:ntA] = blk_tok[c]
        sl = slice(42 * c, 42 * c + 42)
        tm[NQA:] = blk_tok[8][sl]
        tokmap.append(tm)
        x_own[c][:, :ntA] = x[:, blk_tok[c]]
        x_own[c][:, NQA:] = x[:, blk_tok[8][sl]]
        for l in range(DEPTH):
            ker = kers[l]
            # piece A: keys rows [0:ntA], queries cols [0:ntA]
            t1A = ker[iA, jA] * mA + (1.0 - mA)
            T1[c, l, :ntA, :ntA] = t1A
            T1[c, l, ntA:, :] = 1.0
            T1[c, l, :, ntA:NQA] = 1.0
            # piece B: keys rows [0:336] (block-8 packed), q cols [NQA:]
            t1B = ker[iB, jB] * mB + (1.0 - mB)
            T1[c, l, :336, NQA:] = t1B[:, sl]
            T1[c, l, 336:, NQA:] = 1.0
            for h in range(HEADS):
                rp = rpb[l, h]
                t2A = rp[rA, cA] * mA + NEG * (1.0 - mA)
                T2[c, l, h, :ntA, :ntA] = t2A
                T2[c, l, h, ntA:, :ntA] = NEG
                T2[c, l, h, :, ntA:NQA] = 0.0
                t2B = rp[rB, cB] * mB + NEG * (1.0 - mB)
                T2[c, l, h, :336, NQA:] = t2B[:, sl]
                T2[c, l, h, 336:, NQA:] = NEG

    qkv_w = np.asarray(inputs["qkv_w"], np.float32)
    w = {
        "qk_wT": np.ascontiguousarray(qkv_w[:, :512, :].transpose(0, 2, 1)),
        "v_wT": np.ascontiguousarray(qkv_w[:, 512:, :].transpose(0, 2, 1)),
        "qk_b": np.ascontiguousarray(np.asarray(inputs["qkv_b"], np.float32)[:, :512]),
        "v_b": np.ascontiguousarray(np.asarray(inputs["qkv_b"], np.float32)[:, 512:]),
        "proj_wT": np.ascontiguousarray(np.asarray(inputs["proj_w"], np.float32).transpose(0, 2, 1)),
        "proj_b": np.asarray(inputs["proj_b"], np.float32),
        "fc1_wT": np.ascontiguousarray(np.asarray(inputs["fc1_w"], np.float32).transpose(0, 2, 1)),
        "fc1_b": np.asarray(inputs["fc1_b"], np.float32),
        "fc2_wT": np.ascontiguousarray(np.asarray(inputs["fc2_w"], np.float32).transpose(0, 2, 1)),
        "fc2_b": np.asarray(inputs["fc2_b"], np.float32),
        "ln1_w": np.asarray(inputs["ln1_w"], np.float32),
        "ln1_b": np.asarray(inputs["ln1_b"], np.float32),
        "ln2_w": np.asarray(inputs["ln2_w"], np.float32),
        "ln2_b": np.asarray(inputs["ln2_b"], np.float32),
        "normf_w": np.asarray(inputs["normf_w"], np.float32).reshape(1, DIM),
        "normf_b": np.asarray(inputs["normf_b"], np.float32).reshape(1, DIM),
    }
    blk4 = np.zeros((4, 128), np.float32)
    for gidx in range(4):
        blk4[gidx, 32 * gidx:32 * gidx + 32] = 1.0
    w["blk4"] = blk4
    return x_own, T1, T2, w, tokmap


# ag_in layout per rank (flat elems): qk-slice [512,42] then vT-slice [42,256]
AG_QK, AG_VT = 512 * 42, 42 * 256
AG_N = AG_QK + AG_VT


def _build_program(skip_ag=False):
    nc = bacc.Bacc("TRN2", target_bir_lowering=False, debug=False,
                   num_devices=1 if skip_ag else N_CORES)
    dram = {}
    def din(name, shape, dt=F32):
        dram[name] = nc.dram_tensor(name, list(shape), dt, kind="ExternalInput")
        return dram[name]

    din("x_own", (DIM, NTOK))
    din("T1", (DEPTH, NKEY, NTOK))
    din("T2", (DEPTH, HEADS, NKEY, NTOK))
    din("qk_wT", (DEPTH, DIM, 512), F32R); din("v_wT", (DEPTH, DIM, 256), F32R)
    din("qk_b", (DEPTH, 512)); din("v_b", (DEPTH, 256))
    din("proj_wT", (DEPTH, DIM, DIM), F32R); din("proj_b", (DEPTH, DIM))
    din("fc1_wT", (DEPTH, DIM, 4 * DIM), F32R); din("fc1_b", (DEPTH, 4 * DIM))
    din("fc2_wT", (DEPTH, 4 * DIM, DIM), F32R); din("fc2_b", (DEPTH, DIM))
    din("ln1_w", (DEPTH, DIM)); din("ln1_b", (DEPTH, DIM))
    din("ln2_w", (DEPTH, DIM)); din("ln2_b", (DEPTH, DIM))
    din("normf_w", (1, DIM)); din("normf_b", (1, DIM))
    din("blk4", (4, 128))
    y_own = nc.dram_tensor("y_own", [DIM, NTOK], F32, kind="ExternalOutput")
    ag_in = [nc.dram_tensor(f"ag_in{l}", [AG_N], BF16) for l in range(DEPTH)]
    ag_out = [nc.dram_tensor(f"ag_out{l}", [N_CORES * AG_N], BF16,
                             addr_space="Shared") for l in range(DEPTH)]

    def mm(out, lhsT, rhs, start, stop, tp=None):
        kw = {} if tp is None else {"tile_position": tp}
        nc.tensor.matmul(out, lhsT, rhs, start=start, stop=stop, **kw)

    with tile.TileContext(nc) as tc:
        import contextlib
        with contextlib.ExitStack() as ctx:
            ctx.enter_context(nc.allow_low_precision(
                reason="float32r tiles hold fp32 data for 1-cyc/row matmuls"))
            wp = ctx.enter_context(tc.tile_pool(name="wp", bufs=2))
            tpool = ctx.enter_context(tc.tile_pool(name="tp", bufs=2))
            t2p = ctx.enter_context(tc.tile_pool(name="t2p", bufs=3))
            sp = ctx.enter_context(tc.tile_pool(name="sp", bufs=2))
            pp = ctx.enter_context(tc.tile_pool(name="pp", bufs=2))
            rp = ctx.enter_context(tc.tile_pool(name="rp", bufs=1))
            cn = ctx.enter_context(tc.tile_pool(name="cn", bufs=1))
            ps_s = ctx.enter_context(tc.tile_pool(name="ps_s", bufs=3, space="PSUM"))
            ps_sm = ctx.enter_context(tc.tile_pool(name="ps_sm", bufs=1, space="PSUM"))
            ps_at = ctx.enter_context(tc.tile_pool(name="ps_at", bufs=2, space="PSUM"))
            ps_mm = ctx.enter_context(tc.tile_pool(name="ps_mm", bufs=2, space="PSUM"))

            ones_col = cn.tile([128, 1], F32); nc.vector.memset(ones_col, 1.0)
            ones_b = cn.tile([128, 1], BF16)
            nc.vector.tensor_copy(ones_b, ones_col)
            z384 = cn.tile([128, 384], F32); nc.vector.memset(z384, 0.0)
            ones_row = cn.tile([1, 128], F32); nc.vector.memset(ones_row, 1.0)
            eps_t = cn.tile([1, 1], F32); nc.vector.memset(eps_t, EPS)
            blk4 = cn.tile([4, 128], F32)
            nc.sync.dma_start(out=blk4, in_=dram["blk4"].ap())

            # residual stream, feature-major [2][128, NTOK]
            t_res = [rp.tile([128, NTOK], F32, tag=f"t{i}", name=f"t_res{i}") for i in range(2)]
            for i in range(2):
                nc.sync.dma_start(out=t_res[i],
                                  in_=dram["x_own"].ap()[128 * i:128 * (i + 1), :])

            def layernorm(tin, w_ap, b_ap, name, out_dt=F32):
                su1 = ps_mm.tile([1, NTOK], F32, tag="m")
                su2 = ps_mm.tile([1, NTOK], F32, tag="m")
                for k in range(2):
                    sq = sp.tile([128, NTOK], F32, tag="sq")
                    nc.vector.tensor_mul(sq, tin[k], tin[k])
                    mm(su1, ones_col, tin[k], start=(k == 0), stop=(k == 1))
                    mm(su2, ones_col, sq, start=(k == 0), stop=(k == 1))
                mean = sp.tile([1, NTOK], F32, tag="ln_small", bufs=8)
                ex2 = sp.tile([1, NTOK], F32, tag="ln_small", bufs=8)
                nc.vector.tensor_scalar_mul(mean, su1, 1.0 / DIM)
                nc.vector.tensor_scalar_mul(ex2, su2, 1.0 / DIM)
                var = sp.tile([1, NTOK], F32, tag="ln_small", bufs=8)
                nc.vector.tensor_mul(var, mean, mean)
                nc.vector.tensor_sub(var, ex2, var)
                sd = sp.tile([1, NTOK], F32, tag="ln_small", bufs=8)
                nc.scalar.activation(out=sd, in_=var, func=AFT.Sqrt,
                                     bias=eps_t, scale=1.0)
                rstd = sp.tile([1, NTOK], F32, tag="ln_small", bufs=8)
                nc.vector.reciprocal(rstd, sd)
                a = sp.tile([1, NTOK], F32, tag="ln_small", bufs=8)
                nc.vector.tensor_mul(a, mean, rstd)
                bc_r = ps_mm.tile([128, NTOK], F32, tag="m")
                bc_a = ps_mm.tile([128, NTOK], F32, tag="m")
                mm(bc_r, ones_row, rstd, start=True, stop=True)
                mm(bc_a, ones_row, a, start=True, stop=True)
                y = []
                for k in range(2):
                    u = sp.tile([128, NTOK], F32, tag="ln_u")
                    nc.vector.tensor_mul(u, tin[k], bc_r)
                    nc.vector.tensor_sub(u, u, bc_a)
                    yk = sp.tile([128, 512 if out_dt is F32R else NTOK], out_dt, tag=f"{name}{k}")
                    nc.vector.tensor_scalar(out=yk[:, 0:NTOK], in0=u, scalar1=w_ap[k],
                                            scalar2=b_ap[k],
                                            op0=mybir.AluOpType.mult,
                                            op1=mybir.AluOpType.add)
                    y.append(yk)
                return y

            for l in range(DEPTH):
                # --- load layer weights ---
                qkw = [wp.tile([128, 512], F32R, tag="qkw", name="qkw") for _ in range(2)]
                vw = [wp.tile([128, 256], F32R, tag="vw", name="vw") for _ in range(2)]
                pw = [wp.tile([128, 256], F32R, tag="pw", name="pw") for _ in range(2)]
                f1w = [wp.tile([128, 1024], F32R, tag="f1w", name="f1w") for _ in range(2)]
                f2w = [wp.tile([128, 256], F32R, tag="f2w", name="f2w", bufs=10) for _ in range(8)]
                for k in range(2):
                    nc.sync.dma_start(out=qkw[k], in_=dram["qk_wT"].ap()[l, 128 * k:128 * k + 128, :])
                    nc.sync.dma_start(out=vw[k], in_=dram["v_wT"].ap()[l, 128 * k:128 * k + 128, :])
                    nc.sync.dma_start(out=pw[k], in_=dram["proj_wT"].ap()[l, 128 * k:128 * k + 128, :])
                    nc.sync.dma_start(out=f1w[k], in_=dram["fc1_wT"].ap()[l, 128 * k:128 * k + 128, :])
                for k in range(8):
                    nc.sync.dma_start(out=f2w[k], in_=dram["fc2_wT"].ap()[l, 128 * k:128 * k + 128, :])
                lnw = {}
                for nm in ("ln1_w", "ln1_b", "ln2_w", "ln2_b", "qk_b", "proj_b",
                           "fc1_b", "fc2_b"):
                    nparts = {"qk_b": 4, "fc1_b": 8}.get(nm, 2)
                    tl = wp.tile([128, nparts], F32, tag=nm)
                    src = dram[nm].ap()[l, :].rearrange("(a p) -> p a", p=128)
                    nc.sync.dma_start(out=tl, in_=src)
                    lnw[nm] = tl
                vb_bc = wp.tile([128, 256], F32, tag="vb_bc")
                nc.sync.dma_start(
                    out=vb_bc,
                    in_=bass.AP(tensor=dram["v_b"], offset=l * 256,
                                ap=[[0, 128], [1, 256]]))
                t1t = [tpool.tile([128, NTOK], F32, tag="t1", name="t1t", bufs=6) for _ in range(3)]
                for j in range(3):
                    nc.sync.dma_start(out=t1t[j],
                                      in_=dram["T1"].ap()[l, 128 * j:128 * j + 128, :])

                # --- LN1 ---
                y1 = layernorm(t_res, [lnw["ln1_w"][:, k:k + 1] for k in range(2)],
                               [lnw["ln1_b"][:, k:k + 1] for k in range(2)], "y1", out_dt=F32R)

                # --- qk projection (feature-major) ---
                qk_sb = []
                for m in range(4):
                    ps = ps_mm.tile([128, NTOK], F32, tag="m")
                    for k in range(2):
                        mm(ps, qkw[k][:, 128 * m:128 * m + 128], y1[k][:, 0:NTOK],
                           start=(k == 0), stop=(k == 1))
                    sb = sp.tile([128, NTOK], BF16, tag=f"qk{m}")
                    nc.vector.tensor_scalar(out=sb, in0=ps,
                                            scalar1=HD ** -0.5 if m < 2 else 1.0,
                                            scalar2=lnw["qk_b"][:, m:m + 1],
                                            op0=mybir.AluOpType.mult,
                                            op1=mybir.AluOpType.add)
                    qk_sb.append(sb)

                # --- vT (token-major) ---
                vt_sb = []
                for k in range(2):
                    nc.vector.tensor_copy(y1[k][:, NTOK:512], z384[:, 0:512 - NTOK])
                for mt in range(4):
                    npt = min(128, NTOK - 128 * mt)
                    ps = ps_mm.tile([128, 256], F32, tag="m")
                    for k in range(2):
                        mm(ps, y1[k][:, 128 * mt:128 * mt + 128], vw[k],
                           start=(k == 0), stop=(k == 1))
                    sb = sp.tile([128, 256], BF16, tag=f"vt{mt}")
                    nc.vector.tensor_add(sb[:npt, :], ps[:npt, :], vb_bc[:npt, :])
                    vt_sb.append(sb)

                # --- AG of block-8 slice ---
                for m in range(4):
                    nc.sync.dma_start(
                        out=bass.AP(tensor=ag_in[l], offset=m * 128 * 42,
                                    ap=[[42, 128], [1, 42]]),
                        in_=qk_sb[m][:, NQA:NTOK])
                nc.sync.dma_start(
                    out=bass.AP(tensor=ag_in[l], offset=AG_QK,
                                ap=[[256, 32], [1, 256]]),
                    in_=vt_sb[2][96:128, :])
                nc.sync.dma_start(
                    out=bass.AP(tensor=ag_in[l], offset=AG_QK + 32 * 256,
                                ap=[[256, 10], [1, 256]]),
                    in_=vt_sb[3][:10, :])
                if not skip_ag:
                    nc.gpsimd.collective_compute(
                        "AllGather", mybir.AluOpType.bypass,
                        replica_groups=[list(range(N_CORES))],
                        ins=[ag_in[l].ap().opt()], outs=[ag_out[l].ap().opt()])

                # --- gather block-8 K (feature-major) and vT (token-major) ---
                kb = [sp.tile([128, NKEY], BF16, tag=f"kb{i}", name=f"kb{i}") for i in range(2)]
                for i in range(2):
                    nc.vector.tensor_copy(kb[i], z384)
                    nc.sync.dma_start(
                        out=bass.AP(tensor=kb[i].tensor, offset=kb[i].offset,
                                    ap=[kb[i].ap[0], [42, 8], [1, 42]]),
                        in_=bass.AP(tensor=ag_out[l],
                                    offset=(256 + 128 * i) * 42,
                                    ap=[[42, 128], [AG_N, 8], [1, 42]]))
                vb = [sp.tile([128, 256], BF16, tag=f"vb{i}", name=f"vb{i}") for i in range(3)]
                for i in range(3):
                    nc.vector.tensor_copy(vb[i], z384[:, 0:256])
                for r in range(8):
                    row0 = 42 * r
                    left = 42
                    while left > 0:
                        ti, ri = row0 // 128, row0 % 128
                        cnt = min(left, 128 - ri)
                        nc.sync.dma_start(
                            out=vb[ti][ri:ri + cnt, :],
                            in_=bass.AP(tensor=ag_out[l],
                                        offset=AG_N * r + AG_QK + (row0 - 42 * r) * 256,
                                        ap=[[256, cnt], [1, 256]]))
                        row0 += cnt; left -= cnt

                # --- attention ---
                inv4 = [sp.tile([4, NTOK], F32, tag=f"inv{g}", name=f"inv{g}") for g in range(2)]
                attn_sb = [sp.tile([128, NTOK], F32R, tag=f"at{g}", name=f"attn_sb{g}") for g in range(2)]
                for g in range(2):
                    at_ps = ps_at.tile([128, NTOK], F32, tag="at")
                    av_b = ps_at.tile([128, NQB], F32, tag="avb", bufs=1, name="av_b")
                    for hh in range(4):
                        h = 4 * g + hh
                        qt, kt = qk_sb[h // 4], qk_sb[2 + h // 4]
                        hr = slice(32 * (h % 4), 32 * (h % 4) + 32)
                        pj = []
                        for j in range(3):
                            s_ps = ps_s.tile([128, NTOK], F32, tag="s")
                            tp = (32 * (h % 4), 0) if h % 4 == 3 else None
                            mm(s_ps[:, 0:NQA], kt[hr, 128 * j:128 * j + 128],
                               qt[hr, 0:NQA], start=True, stop=True, tp=tp)
                            mm(s_ps[:, NQA:NTOK], kb[h // 4][32 * (h % 4):32 * (h % 4) + 32, 128 * j:128 * j + 128],
                               qt[hr, NQA:NTOK], start=True, stop=True, tp=tp)
                            t2 = t2p.tile([128, NTOK], F32, tag="t2", bufs=6)
                            nc.sync.dma_start(
                                out=t2, in_=dram["T2"].ap()[l, h, 128 * j:128 * j + 128, :])
                            x1 = sp.tile([128, NTOK], F32, tag="x1", bufs=3)
                            nc.vector.tensor_add(x1, s_ps, t2)
                            x2 = sp.tile([128, NTOK], F32, tag="x2", bufs=3)
                            nc.gpsimd.tensor_mul(x2, x1, t1t[j])
                            p = sp.tile([128, NTOK], BF16, tag=f"p{j}", bufs=3)
                            nc.scalar.activation(out=p, in_=x2, func=AFT.Exp)
                            pj.append(p)
                        sm = ps_mm.tile([1, NTOK], F32, tag="m")
                        for j in range(3):
                            mm(sm, ones_b, pj[j], start=(j == 0), stop=(j == 2))
                        invh = sp.tile([1, NTOK], F32, tag="invh", bufs=4, name="invh")
                        nc.vector.reciprocal(invh, sm)
                        nc.sync.dma_start(out=inv4[g][hh:hh + 1, :], in_=invh)
                        tpav = (0, 32 * (h % 4)) if h % 4 == 3 else None
                        for j in range(3):
                            mm(at_ps[hr, 0:NQA], vt_sb[j][:, 32 * h:32 * h + 32],
                               pj[j][:, 0:NQA], start=(j == 0), stop=(j == 2), tp=tpav)
                            mm(av_b[hr, :], vb[j][:, 32 * h:32 * h + 32],
                               pj[j][:, NQA:NTOK], start=(j == 0), stop=(j == 2), tp=tpav)
                    bc = ps_mm.tile([128, NTOK], F32, tag="m")
                    mm(bc, blk4, inv4[g], start=True, stop=True)
                    bc_sb = sp.tile([128, NTOK], F32, tag="bc_sb")
                    nc.scalar.activation(out=bc_sb, in_=bc, func=AFT.Copy)
                    nc.vector.tensor_mul(attn_sb[g][:, 0:NQA], at_ps[:, 0:NQA],
                                         bc_sb[:, 0:NQA])
                    nc.vector.tensor_mul(attn_sb[g][:, NQA:NTOK], av_b,
                                         bc_sb[:, NQA:NTOK])

                # --- proj + residual ---
                for i in range(2):
                    ps = ps_mm.tile([128, NTOK], F32, tag="m")
                    for k in range(2):
                        mm(ps, pw[k][:, 128 * i:128 * i + 128], attn_sb[k],
                           start=(k == 0), stop=(k == 1))
                    pb = sp.tile([128, NTOK], F32, tag="pb")
                    nc.vector.tensor_scalar(out=pb, in0=ps,
                                            scalar1=lnw["proj_b"][:, i:i + 1],
                                            scalar2=None,
                                            op0=mybir.AluOpType.add)
                    nc.vector.tensor_add(t_res[i], t_res[i], pb)

                # --- LN2 + MLP ---
                y2 = layernorm(t_res, [lnw["ln2_w"][:, k:k + 1] for k in range(2)],
                               [lnw["ln2_b"][:, k:k + 1] for k in range(2)], "y2", out_dt=F32R)
                gs = []
                for m in range(8):
                    ps = ps_mm.tile([128, NTOK], F32, tag="m")
                    for k in range(2):
                        mm(ps, f1w[k][:, 128 * m:128 * m + 128], y2[k][:, 0:NTOK],
                           start=(k == 0), stop=(k == 1))
                    gm = sp.tile([128, NTOK], F32R, tag=f"g{m}")
                    nc.scalar.activation(out=gm, in_=ps, func=AFT.Gelu,
                                         bias=lnw["fc1_b"][:, m:m + 1], scale=1.0)
                    gs.append(gm)
                for i in range(2):
                    ps = ps_mm.tile([128, NTOK], F32, tag="m")
                    for k in range(8):
                        mm(ps, f2w[k][:, 128 * i:128 * i + 128], gs[k],
                           start=(k == 0), stop=(k == 7))
                    fb = sp.tile([128, NTOK], F32, tag="fb")
                    nc.vector.tensor_scalar(out=fb, in0=ps,
                                            scalar1=lnw["fc2_b"][:, i:i + 1],
                                            scalar2=None,
                                            op0=mybir.AluOpType.add)
                    nc.vector.tensor_add(t_res[i], t_res[i], fb)

            # --- final LN + output ---
            nfw = cn.tile([128, 2], F32); nfb = cn.tile([128, 2], F32)
            nc.sync.dma_start(out=nfw, in_=dram["normf_w"].ap()[0, :].rearrange("(a p) -> p a", p=128))
            nc.sync.dma_start(out=nfb, in_=dram["normf_b"].ap()[0, :].rearrange("(a p) -> p a", p=128))
            yf = layernorm(t_res, [nfw[:, k:k + 1] for k in range(2)],
                           [nfb[:, k:k + 1] for k in range(2)], "yf")
            for k in range(2):
                nc.sync.dma_start(out=y_own.ap()[128 * k:128 * k + 128, :], in_=yf[k])

    nc.compile()
    return nc


def kernel(**inputs):
    x_own, T1, T2, w, tokmap = _host_prep(inputs)
    if "prog" not in _cache:
        _cache["prog"] = _build_program()
    nc = _cache["prog"]
    in_maps = []
    for c in range(N_CORES):
        m = {"x_own": x_own[c], "T1": T1[c], "T2": T2[c]}
        for k, v in w.items():
            m[k] = v
        in_maps.append(m)
    res = run_bass_kernel_spmd(nc, in_maps, core_ids=list(range(N_CORES)),
                               trace=bool(os.environ.get("BASS_TRACE_RUN")))
    _cache["last_results"] = res
    out = np.zeros((DIM, H * W), np.float32)
    for c in range(N_CORES):
        yc = res.results[c]["y_own"]
        valid = tokmap[c] >= 0
        out[:, tokmap[c][valid]] = yc[:, valid]
    return out.reshape(1, DIM, H, W)



# revision 29
# speedup vs baseline: 1.4146x; 1.4146x over previous
"""NATTEN-style dilated neighborhood-attention transformer on 8 trn2 cores.

Design (v2):
- Dilation-3 NA factorizes into 9 independent (row-class s, col-class r)
  blocks; in class space each block is a dense 16 x nc map with a k=13, d=1
  neighborhood.  Sharding: core c owns class-block c (c=0..7); block 8's
  queries are split 42/core (2 class-rows each).  All residual-stream ops
  (LN, QKV, proj, MLP) are token-local; only block-8 attention needs an
  AllGather of each rank's 42-token k/vT slice (~64KB/rank) per layer.
- Attention math per block: masked-dense scores S[k, q] (keys on
  partitions) accumulated in PSUM on top of a host-precomputed additive
  table T2 = rpb_rel*M - 30*(1-M) (per layer/head, bf16, streamed once per
  layer in a single DMA) via an identity-matmul; then one fused
  scalar_tensor_tensor multiply by T1 = ker*M + (1-M) (bf16, resident all
  layers), one Exp per head over the j-merged [128, 3*394] tile, softmax
  denominators via selector-column matmuls accumulated into a [4, NTOK]
  PSUM tile, AV with token-major vT, normalize via a block-broadcast
  matmul + one multiply per 4-head group.
- All weights bf16, packed host-side into one DRAM tensor, loaded once
  into SBUF (resident for all 6 layers); biases/LN params packed; v-bias
  folded into the proj bias; the q-scale folded into qkv weights.  LN uses
  matmul reductions over the f32r residual stream, rstd =
  exp(-0.5*ln(var+eps)) so attention-path ACT ops share one activation
  table set, and broadcast matmuls + fused STT ops for the apply step.
"""
import numpy as np
import sys, os
sys.path.insert(0, "/opt/trn_rl_repo")

import ml_dtypes
import concourse.bass as bass
import concourse.tile as tile
from concourse import mybir, bacc
from concourse.bass_utils import run_bass_kernel_spmd

F32 = mybir.dt.float32
F32R = mybir.dt.float32r
BF16 = mybir.dt.bfloat16
AFT = mybir.ActivationFunctionType
ALU = mybir.AluOpType

KSZ, DIL, SIGMA, SC, EPS = 13, 3, 9.0, 0.1, 1e-5
DEPTH, DIM, HEADS = 6, 256, 8
HD = DIM // HEADS
H, W = 48, 64
NU = H // DIL                      # 16 class rows
NCOLS = [22, 21, 21]               # class cols for r=0,1,2
NB = KSZ // 2                      # 6
NTOK = 394                         # 352 (block, padded) + 42 (block-8 slice)
NQA, NQB = 352, 42
NKEY = 384                         # 3 k-tiles of 128
N_CORES = 8
NEG = -30.0
WCOLS = 6144                       # packed weight cols per layer (bf16)
BCOLS = 24                         # packed bias cols per layer (f32)
NROW = (4 * DEPTH + 2) * 128       # lnrow cols

_cache = {}


def _win_start(L):
    return np.clip(np.arange(L) - NB, 0, L - KSZ)


def _host_prep(inputs):
    """Precompute per-core input tensors (numpy)."""
    x = np.asarray(inputs["x"], np.float32).reshape(DIM, H * W)
    rpb = np.asarray(inputs["rpb"], np.float32)
    lr_m = np.asarray(inputs["lr_m"], np.float32)
    crd = np.arange(KSZ, dtype=np.float32)
    g = np.exp(-((crd[None, :] - KSZ // 2) ** 2 + (crd[:, None] - KSZ // 2) ** 2)
               / (2.0 * SIGMA ** 2))
    kers = [g + lr_m[l] / SC for l in range(DEPTH)]          # (13,13) per layer

    # block token coords: block b=3s+r -> flat hw indices, (u,qc) row-major
    blk_tok = []
    for s in range(3):
        for r in range(3):
            nc_ = NCOLS[r]
            u, qc = np.meshgrid(np.arange(NU), np.arange(nc_), indexing="ij")
            blk_tok.append(((3 * u + s) * W + (3 * qc + r)).reshape(-1))

    # per-block T1/T2 in class space, [nkeys=16*nc, nq=16*nc]
    def block_tables(nc_):
        key = ("bt", nc_)
        if key not in _cache:
            su, sw = _win_start(NU), _win_start(nc_)
            KU, KC, U, QC = np.meshgrid(np.arange(NU), np.arange(nc_),
                                        np.arange(NU), np.arange(nc_), indexing="ij")
            m = ((KU >= su[U]) & (KU <= su[U] + KSZ - 1)
                 & (KC >= sw[QC]) & (KC <= sw[QC] + KSZ - 1))
            i = np.where(m, KU - su[U], 0)
            j = np.where(m, KC - sw[QC], 0)
            rr = np.where(m, KU - U + KSZ - 1, 0)
            cc = np.where(m, KC - QC + KSZ - 1, 0)
            _cache[key] = (m.reshape(NU * nc_, NU * nc_), i.reshape(NU * nc_, -1),
                           j.reshape(NU * nc_, -1), rr.reshape(NU * nc_, -1),
                           cc.reshape(NU * nc_, -1))
        return _cache[key]

    T1 = np.zeros((N_CORES, DEPTH, NKEY, NTOK), np.float32)
    T2 = np.zeros((N_CORES, DEPTH, HEADS, NKEY, NTOK), np.float32)
    x_own = np.zeros((N_CORES, DIM, NTOK), np.float32)
    tokmap = []   # per core: global flat-hw index per col (or -1 pad)
    for c in range(N_CORES):
        ncA = NCOLS[c % 3]
        ntA = NU * ncA
        mA, iA, jA, rA, cA = block_tables(ncA)
        mB, iB, jB, rB, cB = block_tables(21)
        tm = np.full(NTOK, -1, np.int64)
        tm[:ntA] = blk_tok[c]
        sl = slice(42 * c, 42 * c + 42)
        tm[NQA:] = blk_tok[8][sl]
        tokmap.append(tm)
        x_own[c][:, :ntA] = x[:, blk_tok[c]]
        x_own[c][:, NQA:] = x[:, blk_tok[8][sl]]
        for l in range(DEPTH):
            ker = kers[l]
            # piece A: keys rows [0:ntA], queries cols [0:ntA]
            t1A = ker[iA, jA] * mA + (1.0 - mA)
            T1[c, l, :ntA, :ntA] = t1A
            T1[c, l, ntA:, :] = 1.0
            T1[c, l, :, ntA:NQA] = 1.0
            # piece B: keys rows [0:336] (block-8 packed), q cols [NQA:]
            t1B = ker[iB, jB] * mB + (1.0 - mB)
            T1[c, l, :336, NQA:] = t1B[:, sl]
            T1[c, l, 336:, NQA:] = 1.0
            for h in range(HEADS):
                rp = rpb[l, h]
                t2A = rp[rA, cA] * mA + NEG * (1.0 - mA)
                T2[c, l, h, :ntA, :ntA] = t2A
                T2[c, l, h, ntA:, :ntA] = NEG
                T2[c, l, h, :, ntA:NQA] = 0.0
                t2B = rp[rB, cB] * mB + NEG * (1.0 - mB)
                T2[c, l, h, :336, NQA:] = t2B[:, sl]
                T2[c, l, h, 336:, NQA:] = NEG

    BF = ml_dtypes.bfloat16
    # t1pack [C, DEPTH, 128, 3, NTOK]: key = 128*j + p
    t1pack = np.ascontiguousarray(
        T1.reshape(N_CORES, DEPTH, 3, 128, NTOK).transpose(0, 1, 3, 2, 4)
    ).astype(BF)
    # t2pack [C, DEPTH, 128, HEADS, 3, NTOK]
    t2pack = np.ascontiguousarray(
        T2.reshape(N_CORES, DEPTH, HEADS, 3, 128, NTOK).transpose(0, 1, 4, 2, 3, 5)
    ).astype(BF)

    qkv_w = np.asarray(inputs["qkv_w"], np.float32).copy()
    qkv_b = np.asarray(inputs["qkv_b"], np.float32).copy()
    qkv_w[:, :256, :] *= HD ** -0.5          # fold q scale
    qkv_b[:, :256] *= HD ** -0.5
    proj_w = np.asarray(inputs["proj_w"], np.float32)
    proj_b = (np.asarray(inputs["proj_b"], np.float32)
              + np.einsum("lij,lj->li", proj_w, qkv_b[:, 512:]))  # fold v bias
    fc1_w = np.asarray(inputs["fc1_w"], np.float32)
    fc2_w = np.asarray(inputs["fc2_w"], np.float32)

    # wpack [128, DEPTH, WCOLS] bf16; per-layer block:
    # qkw k0(512) k1(512) | vw k0(256) k1(256) | pw k0(256) k1(256)
    # | f1w k0(1024) k1(1024) | f2w 0..7 (2048)
    wpack = np.zeros((128, DEPTH, WCOLS), np.float32)
    for l in range(DEPTH):
        qk_wT = qkv_w[l, :512, :].T          # [256, 512]
        v_wT = qkv_w[l, 512:, :].T           # [256, 256]
        p_wT = proj_w[l].T                   # [256, 256]
        f1_wT = fc1_w[l].T                   # [256, 1024]
        f2_wT = fc2_w[l].T                   # [1024, 256]
        for k in range(2):
            wpack[:, l, 512 * k:512 * k + 512] = qk_wT[128 * k:128 * k + 128]
            wpack[:, l, 1024 + 256 * k:1024 + 256 * k + 256] = v_wT[128 * k:128 * k + 128]
            wpack[:, l, 1536 + 256 * k:1536 + 256 * k + 256] = p_wT[128 * k:128 * k + 128]
            wpack[:, l, 2048 + 1024 * k:2048 + 1024 * k + 1024] = f1_wT[128 * k:128 * k + 128]
        for k in range(8):
            wpack[:, l, 4096 + 256 * k:4096 + 256 * k + 256] = f2_wT[128 * k:128 * k + 128]

    # bcols [128, DEPTH*24 + 4] f32; per layer:
    # ln1_w(2) ln1_b(2) ln2_w(2) ln2_b(2) qk_b(4) proj_b'(2) fc1_b(8) fc2_b(2)
    bcols = np.zeros((128, DEPTH * BCOLS + 4), np.float32)
    for l in range(DEPTH):
        B = BCOLS * l
        for k in range(2):
            bcols[:, B + k] = np.asarray(inputs["ln1_w"], np.float32)[l, 128 * k:128 * k + 128]
            bcols[:, B + 2 + k] = np.asarray(inputs["ln1_b"], np.float32)[l, 128 * k:128 * k + 128]
            bcols[:, B + 4 + k] = np.asarray(inputs["ln2_w"], np.float32)[l, 128 * k:128 * k + 128]
            bcols[:, B + 6 + k] = np.asarray(inputs["ln2_b"], np.float32)[l, 128 * k:128 * k + 128]
        for m in range(4):
            bcols[:, B + 8 + m] = qkv_b[l, 128 * m:128 * m + 128]
        for i in range(2):
            bcols[:, B + 12 + i] = proj_b[l, 128 * i:128 * i + 128]
        for m in range(8):
            bcols[:, B + 14 + m] = np.asarray(inputs["fc1_b"], np.float32)[l, 128 * m:128 * m + 128]
        for i in range(2):
            bcols[:, B + 22 + i] = np.asarray(inputs["fc2_b"], np.float32)[l, 128 * i:128 * i + 128]
    for k in range(2):
        bcols[:, DEPTH * BCOLS + k] = np.asarray(inputs["normf_w"], np.float32)[128 * k:128 * k + 128]
        bcols[:, DEPTH * BCOLS + 2 + k] = np.asarray(inputs["normf_b"], np.float32)[128 * k:128 * k + 128]

    # lnrow [(DEPTH+1), 1024] bf16: per-layer row blocks for the folded
    # broadcast lhsT: w rows at 256*which+128*k, -b rows at 512+256*which+128*k.
    # Row DEPTH holds normf (which=0 slots).
    lnrow = np.zeros((DEPTH + 1, 1024), np.float32)
    for l in range(DEPTH):
        for k in range(2):
            lnrow[l, 128 * k:128 * k + 128] = np.asarray(
                inputs["ln1_w"], np.float32)[l, 128 * k:128 * k + 128]
            lnrow[l, 512 + 128 * k:512 + 128 * k + 128] = -np.asarray(
                inputs["ln1_b"], np.float32)[l, 128 * k:128 * k + 128]
            lnrow[l, 256 + 128 * k:256 + 128 * k + 128] = np.asarray(
                inputs["ln2_w"], np.float32)[l, 128 * k:128 * k + 128]
            lnrow[l, 768 + 128 * k:768 + 128 * k + 128] = -np.asarray(
                inputs["ln2_b"], np.float32)[l, 128 * k:128 * k + 128]
    for k in range(2):
        lnrow[DEPTH, 128 * k:128 * k + 128] = np.asarray(
            inputs["normf_w"], np.float32)[128 * k:128 * k + 128]
        lnrow[DEPTH, 512 + 128 * k:512 + 128 * k + 128] = -np.asarray(
            inputs["normf_b"], np.float32)[128 * k:128 * k + 128]

    # cpack [128, 144] bf16: identity(128) + sel4(16)
    cpack = np.zeros((128, 144), np.float32)
    cpack[:, :128] = np.eye(128, dtype=np.float32)
    for hh in range(4):
        cpack[:, 128 + 4 * hh + hh] = 1.0
    blk4 = np.zeros((4, 128), np.float32)
    for gidx in range(4):
        blk4[gidx, 32 * gidx:32 * gidx + 32] = 1.0

    w = {
        "wpack": wpack.astype(BF), "bcols": bcols,
        "lnrow": lnrow.astype(BF),
        "cpack": cpack.astype(BF), "blk4": blk4,
        "rowc": np.ones((1, 128), np.float32),
        "colc": np.full((128, 1), 1.0 / DIM, np.float32),
    }
    return x_own, t1pack, t2pack, w, tokmap


# ag_in layout per rank (flat elems): qk-slice [512,42] then vT-slice [42,256]
AG_QK, AG_VT = 512 * 42, 42 * 256
AG_N = AG_QK + AG_VT


def _build_program(skip_ag=False):
    nc = bacc.Bacc("TRN2", target_bir_lowering=False, debug=False,
                   num_devices=1 if skip_ag else N_CORES)
    dram = {}
    def din(name, shape, dt=F32):
        dram[name] = nc.dram_tensor(name, list(shape), dt, kind="ExternalInput")
        return dram[name]

    din("x_own", (DIM, NTOK), F32R)
    din("t1pack", (DEPTH, 128, 3, NTOK), BF16)
    din("t2pack", (DEPTH, 128, HEADS, 3, NTOK), BF16)
    din("wpack", (128, DEPTH, WCOLS), BF16)
    din("bcols", (128, DEPTH * BCOLS + 4))
    din("lnrow", (DEPTH + 1, 1024), BF16)
    din("cpack", (128, 144), BF16)
    din("blk4", (4, 128), F32R)
    din("rowc", (1, 128), F32R)
    din("colc", (128, 1), F32R)
    y_own = nc.dram_tensor("y_own", [DIM, NTOK], F32, kind="ExternalOutput")
    ag_in = [nc.dram_tensor(f"ag_in{l}", [AG_N], BF16) for l in range(DEPTH)]
    ag_out = [nc.dram_tensor(f"ag_out{l}", [N_CORES * AG_N], BF16,
                             addr_space="Shared") for l in range(DEPTH)]

    def mm(out, lhsT, rhs, start, stop, tp=None, skip=False):
        kw = {}
        if tp is not None:
            kw["tile_position"] = tp
        if skip:
            kw["skip_group_check"] = True
        nc.tensor.matmul(out, lhsT, rhs, start=start, stop=stop, **kw)

    with tile.TileContext(nc) as tc:
        import contextlib
        with contextlib.ExitStack() as ctx:
            ctx.enter_context(nc.allow_low_precision(
                reason="bf16 weights/activations; f32r residual stream"))
            cn = ctx.enter_context(tc.tile_pool(name="cn", bufs=1))
            t2p = ctx.enter_context(tc.tile_pool(name="t2p", bufs=2))
            sp = ctx.enter_context(tc.tile_pool(name="sp", bufs=2))
            x2p = ctx.enter_context(tc.tile_pool(name="x2p", bufs=3))
            pjp = ctx.enter_context(tc.tile_pool(name="pjp", bufs=3))
            ps_s = ctx.enter_context(tc.tile_pool(name="ps_s", bufs=3, space="PSUM"))
            ps_at = ctx.enter_context(tc.tile_pool(name="ps_at", bufs=2, space="PSUM"))
            ps_sm = ctx.enter_context(tc.tile_pool(name="ps_sm", bufs=1, space="PSUM"))
            ps_mm = ctx.enter_context(tc.tile_pool(name="ps_mm", bufs=2, space="PSUM"))

            # ---- persistent tiles / constants ----
            wres = [cn.tile([128, WCOLS], BF16, name=f"wres{l}") for l in range(DEPTH)]
            t1res = [cn.tile([128, 3 * NTOK], BF16, name=f"t1res{l}") for l in range(DEPTH)]
            bcol = cn.tile([128, DEPTH * BCOLS + 4], F32, name="bcol")

            cpk = cn.tile([128, 144], BF16, name="cpk")
            blk4 = cn.tile([4, 128], F32R, name="blk4")
            ones_row = cn.tile([1, 128], F32R, name="ones_row")
            ones_sc = cn.tile([128, 1], F32R, name="ones_sc")
            eps_t = cn.tile([1, 1], F32, name="eps_t")
            nc.sync.dma_start(out=ones_row, in_=dram["rowc"].ap())
            nc.sync.dma_start(out=ones_sc, in_=dram["colc"].ap())
            nc.vector.memset(eps_t, EPS)

            # residual stream, feature-major [2][128, NTOK]
            t_res = [cn.tile([128, NTOK], F32R, name=f"t_res{i}") for i in range(2)]
            for i in range(2):
                nc.sync.dma_start(out=t_res[i],
                                  in_=dram["x_own"].ap()[128 * i:128 * (i + 1), :])
            nc.sync.dma_start(out=bcol, in_=dram["bcols"].ap())
            nc.sync.dma_start(out=cpk, in_=dram["cpack"].ap())
            nc.sync.dma_start(out=blk4, in_=dram["blk4"].ap())
            ident = cpk[:, 0:128]

            def wres_load(l):
                nc.sync.dma_start(
                    out=wres[l],
                    in_=bass.AP(tensor=dram["wpack"], offset=WCOLS * l,
                                ap=[[DEPTH * WCOLS, 128], [1, WCOLS]]))

            def t1_load(l):
                nc.sync.dma_start(
                    out=t1res[l],
                    in_=bass.AP(tensor=dram["t1pack"],
                                offset=l * 128 * 3 * NTOK,
                                ap=[[3 * NTOK, 128], [1, 3 * NTOK]]))

            def ln_load(l):
                t = sp.tile([1, 1024], BF16, tag="lnr", bufs=2, name=f"lnr_{l}")
                nc.sync.dma_start(out=t, in_=dram["lnrow"].ap()[l:l + 1, :])
                return t

            def t2_load(l):
                t = t2p.tile([128, HEADS * 3 * NTOK], BF16, tag="t2", name=f"t2_{l}")
                nc.sync.dma_start(
                    out=t,
                    in_=bass.AP(tensor=dram["t2pack"],
                                offset=l * 128 * HEADS * 3 * NTOK,
                                ap=[[HEADS * 3 * NTOK, 128], [1, HEADS * 3 * NTOK]]))
                return t

            wres_load(0)
            t1_load(0)
            lncur = ln_load(0)
            t2cur = t2_load(0)

            def layernorm(lidx, which, out_dt=BF16, out_w=None):
                """which: 0=ln1, 1=ln2, 2=normf. Returns 2 feature-half tiles.
                y_k = (t_k * w_k - (w_k (x) mean - b_k (x) sd)) * (1 (x) rstd)
                """
                lnr = lncur
                if which == 2:
                    wc = [bcol[:, DEPTH * BCOLS + k:DEPTH * BCOLS + k + 1] for k in range(2)]
                    wsel = 0
                else:
                    B = BCOLS * lidx
                    wc = [bcol[:, B + 4 * which + k:B + 4 * which + k + 1] for k in range(2)]
                    wsel = which
                wr = [(lnr[0:1, 256 * wsel + 128 * k:256 * wsel + 128 * k + 128],
                       lnr[0:1, 512 + 256 * wsel + 128 * k:512 + 256 * wsel + 128 * k + 128])
                      for k in range(2)]
                su1 = ps_mm.tile([1, NTOK], F32, tag="m")
                su2 = ps_mm.tile([1, NTOK], F32, tag="m")
                sqs = []
                for k in range(2):
                    sq = sp.tile([128, NTOK], F32R, tag="sq", bufs=2)
                    nc.gpsimd.tensor_mul(sq, t_res[k], t_res[k])
                    sqs.append(sq)
                for k in range(2):
                    mm(su1, ones_sc, t_res[k], start=(k == 0), stop=(k == 1))
                for k in range(2):
                    mm(su2, ones_sc, sqs[k], start=(k == 0), stop=(k == 1))
                mean_sb = sp.tile([1, NTOK], BF16, tag="lns", bufs=5)
                nc.vector.tensor_copy(mean_sb, su1)
                m2 = sp.tile([1, NTOK], F32R, tag="lns", bufs=5)
                nc.vector.scalar_tensor_tensor(out=m2, in0=su1, scalar=1.0,
                                               in1=mean_sb, op0=ALU.bypass,
                                               op1=ALU.mult)
                var = sp.tile([1, NTOK], F32, tag="lns", bufs=5)
                nc.vector.scalar_tensor_tensor(out=var, in0=su2, scalar=1.0,
                                               in1=m2, op0=ALU.bypass, op1=ALU.subtract)
                sd = sp.tile([1, NTOK], BF16, tag="lns", bufs=5)
                nc.scalar.activation(out=sd, in_=var, func=AFT.Sqrt,
                                     bias=eps_t, scale=1.0)
                rstd = sp.tile([1, NTOK], F32R, tag="lns", bufs=5)
                nc.vector.reciprocal(rstd, sd)
                bc_r = sp.tile([128, NTOK], F32R, tag="bcr", bufs=2)
                nc.gpsimd.partition_broadcast(bc_r, rstd, channels=128)
                y = []
                for k in range(2):
                    bc_f = ps_mm.tile([128, NTOK], F32, tag="m")
                    mm(bc_f, wr[k][0], mean_sb, start=True, stop=False)
                    mm(bc_f, wr[k][1], sd, start=False, stop=True)
                    u = sp.tile([128, NTOK], BF16, tag="ln_u", bufs=2)
                    nc.vector.scalar_tensor_tensor(out=u, in0=t_res[k], scalar=wc[k],
                                                   in1=bc_f, op0=ALU.mult, op1=ALU.subtract)
                    wy = out_w if out_w is not None else NTOK
                    yk = sp.tile([128, wy], out_dt, tag=f"y{which}{k}",
                                 bufs=1 if which == 2 else 2)
                    nc.gpsimd.tensor_mul(yk[:, 0:NTOK], u, bc_r)
                    y.append(yk)
                return y

            def preload(func, anchor):
                junk = sp.tile([1, 1], F32, tag="junk", bufs=1)
                nc.scalar.activation(out=junk, in_=anchor, func=func)

            for l in range(DEPTH):
                B = BCOLS * l
                wr_l = wres[l]

                # --- LN1 ---  (y1 tiles 512 wide; tail cols junk, outputs unread)
                y1 = layernorm(l, 0, out_dt=BF16, out_w=512)

                # --- qk projection (feature-major, single merged tile) ---
                qk_sb = sp.tile([128, 4 * NTOK], BF16, tag="qk", bufs=1)
                for m in range(4):
                    pool = ps_mm if m % 2 == 0 else ps_at
                    ps = pool.tile([128, NTOK], F32, tag="m" if m % 2 == 0 else "at")
                    for k in range(2):
                        mm(ps, wr_l[:, 512 * k + 128 * m:512 * k + 128 * m + 128],
                           y1[k][:, 0:NTOK], start=(k == 0), stop=(k == 1))
                    nc.vector.tensor_scalar(out=qk_sb[:, NTOK * m:NTOK * (m + 1)],
                                       in0=ps,
                                       scalar1=bcol[:, B + 8 + m:B + 9 + m],
                                       scalar2=None, op0=ALU.add)

                # --- vT (token-major, single merged tile [128, 4*256]) ---
                vtt = sp.tile([128, 4 * 256], BF16, tag="vt", bufs=1)
                for mt in range(4):
                    npt = min(128, NTOK - 128 * mt)
                    pool = ps_mm if mt % 2 == 0 else ps_at
                    ps = pool.tile([128, 256], F32, tag="m" if mt % 2 == 0 else "at")
                    for k in range(2):
                        mm(ps, y1[k][:, 128 * mt:128 * mt + 128],
                           wr_l[:, 1024 + 256 * k:1024 + 256 * k + 256],
                           start=(k == 0), stop=(k == 1))
                    nc.vector.tensor_copy(vtt[:npt, 256 * mt:256 * mt + 256],
                                     ps[:npt, :])

                # --- AG of block-8 slice ---
                nc.sync.dma_start(
                    out=bass.AP(tensor=ag_in[l], offset=0,
                                ap=[[42, 128], [128 * 42, 4], [1, 42]]),
                    in_=bass.AP(tensor=qk_sb.tensor, offset=qk_sb.offset + NQA,
                                ap=[qk_sb.ap[0], [NTOK, 4], [1, 42]]))
                nc.sync.dma_start(
                    out=bass.AP(tensor=ag_in[l], offset=AG_QK,
                                ap=[[256, 32], [1, 256]]),
                    in_=vtt[96:128, 512:768])
                nc.sync.dma_start(
                    out=bass.AP(tensor=ag_in[l], offset=AG_QK + 32 * 256,
                                ap=[[256, 10], [1, 256]]),
                    in_=vtt[0:10, 768:1024])
                if not skip_ag:
                    nc.gpsimd.collective_compute(
                        "AllGather", ALU.bypass,
                        replica_groups=[list(range(N_CORES))],
                        ins=[ag_in[l].ap().opt()], outs=[ag_out[l].ap().opt()])

                # --- gather block-8 K (feature-major) and vT (token-major) ---
                kb = [sp.tile([128, NKEY], BF16, tag=f"kb{i}", name=f"kb{i}")
                      for i in range(2)]
                for i in range(2):
                    nc.sync.dma_start(
                        out=bass.AP(tensor=kb[i].tensor, offset=kb[i].offset,
                                    ap=[kb[i].ap[0], [42, 8], [1, 42]]),
                        in_=bass.AP(tensor=ag_out[l],
                                    offset=(256 + 128 * i) * 42,
                                    ap=[[42, 128], [AG_N, 8], [1, 42]]))
                vb = [sp.tile([128, 256], BF16, tag=f"vb{i}", name=f"vb{i}")
                      for i in range(3)]
                for r in range(8):
                    row0 = 42 * r
                    left = 42
                    while left > 0:
                        ti, ri = row0 // 128, row0 % 128
                        cnt = min(left, 128 - ri)
                        nc.sync.dma_start(
                            out=vb[ti][ri:ri + cnt, :],
                            in_=bass.AP(tensor=ag_out[l],
                                        offset=AG_N * r + AG_QK + (row0 - 42 * r) * 256,
                                        ap=[[256, cnt], [1, 256]]))
                        row0 += cnt; left -= cnt

                # prefetch next layer's tables/weights (behind the gathers
                # in the DMA FIFO so they don't stall this layer)
                if l + 1 < DEPTH:
                    t2nxt = t2_load(l + 1)
                    wres_load(l + 1)
                    t1_load(l + 1)
                    lnnxt = ln_load(l + 1)
                else:
                    t2nxt = None
                    lnnxt = ln_load(DEPTH)

                # --- attention ---
                attn_sb = [sp.tile([128, NTOK], BF16, tag=f"at{g}", name=f"attn_sb{g}")
                           for g in range(2)]
                for g in range(2):
                    at_ps = ps_at.tile([128, NTOK], F32, tag="at")
                    sm = ps_sm.tile([4, NTOK], F32, tag="sm")

                    def emit_front(hh):
                        h = 4 * g + hh
                        hr = slice(32 * hh, 32 * hh + 32)
                        tp = (96, 0) if hh == 3 else None
                        x2 = x2p.tile([128, 3 * NTOK], F32, tag="x2", name="x2")
                        for j in range(3):
                            s_ps = ps_s.tile([128, NTOK], F32, tag="s")
                            # T2 first (start), then S_A, S_B accumulate
                            mm(s_ps, ident,
                               t2cur[:, (h * 3 + j) * NTOK:(h * 3 + j + 1) * NTOK],
                               start=True, stop=False, skip=True)
                            mm(s_ps[:, 0:NQA],
                               qk_sb[hr, NTOK * (2 + g) + 128 * j:NTOK * (2 + g) + 128 * j + 128],
                               qk_sb[hr, NTOK * g:NTOK * g + NQA],
                               start=False, stop=False, tp=tp, skip=True)
                            mm(s_ps[:, NQA:NTOK],
                               kb[g][hr, 128 * j:128 * j + 128],
                               qk_sb[hr, NTOK * g + NQA:NTOK * (g + 1)],
                               start=False, stop=True, tp=tp, skip=True)
                            # x2 = (S + T2) * T1   (DVE: only engine with PSUM reads)
                            nc.vector.scalar_tensor_tensor(
                                out=x2[:, NTOK * j:NTOK * (j + 1)], in0=s_ps,
                                scalar=1.0,
                                in1=t1res[l][:, j * NTOK:(j + 1) * NTOK],
                                op0=ALU.bypass, op1=ALU.mult)
                        pj = pjp.tile([128, 3 * NTOK], BF16, tag="pj", name="pj")
                        nc.scalar.activation(out=pj, in_=x2, func=AFT.Exp)
                        return pj

                    def emit_back(hh, pj):
                        h = 4 * g + hh
                        hr = slice(32 * hh, 32 * hh + 32)
                        # denominators: sel-column matmuls into [4, NTOK]
                        for j in range(3):
                            mm(sm, cpk[:, 128 + 4 * hh:128 + 4 * hh + 4],
                               pj[:, NTOK * j:NTOK * (j + 1)],
                               start=(hh == 0 and j == 0), stop=(hh == 3 and j == 2),
                               skip=True)
                        # AV
                        tpav = (0, 96) if hh == 3 else None
                        for j in range(3):
                            mm(at_ps[hr, 0:NQA], vtt[:, 256 * j + 32 * h:256 * j + 32 * h + 32],
                               pj[:, NTOK * j:NTOK * j + NQA],
                               start=(j == 0), stop=(j == 2), tp=tpav, skip=True)
                            mm(at_ps[hr, NQA:NTOK], vb[j][:, 32 * h:32 * h + 32],
                               pj[:, NTOK * j + NQA:NTOK * (j + 1)],
                               start=(j == 0), stop=(j == 2), tp=tpav, skip=True)

                    from collections import deque
                    pend = deque()
                    for hh in range(4):
                        pj = emit_front(hh)
                        pend.append((hh, pj))
                        if len(pend) > 2:
                            emit_back(*pend.popleft())
                    while pend:
                        emit_back(*pend.popleft())
                    inv4 = sp.tile([4, NTOK], F32R, tag="inv", bufs=2)
                    nc.vector.reciprocal(inv4, sm)
                    bc = ps_mm.tile([128, NTOK], F32, tag="m")
                    mm(bc, blk4, inv4, start=True, stop=True)
                    bc_sb = sp.tile([128, NTOK], F32R, tag="bc_sb", bufs=2)
                    nc.scalar.activation(out=bc_sb, in_=bc, func=AFT.Copy)
                    nc.vector.scalar_tensor_tensor(out=attn_sb[g], in0=at_ps,
                                                   scalar=1.0, in1=bc_sb,
                                                   op0=ALU.bypass, op1=ALU.mult)

                # --- proj + residual ---
                for i in range(2):
                    ps = ps_mm.tile([128, NTOK], F32, tag="m")
                    for k in range(2):
                        mm(ps, wr_l[:, 1536 + 256 * k + 128 * i:1536 + 256 * k + 128 * i + 128],
                           attn_sb[k], start=(k == 0), stop=(k == 1))
                    nc.vector.scalar_tensor_tensor(
                        out=t_res[i], in0=ps, scalar=bcol[:, B + 12 + i:B + 13 + i],
                        in1=t_res[i], op0=ALU.add, op1=ALU.add)

                # --- LN2 + MLP ---
                y2 = layernorm(l, 1, out_dt=BF16)
                gs = []
                for m in range(8):
                    ps = ps_at.tile([128, NTOK], F32, tag="at")
                    for k in range(2):
                        mm(ps, wr_l[:, 2048 + 1024 * k + 128 * m:2048 + 1024 * k + 128 * m + 128],
                           y2[k][:, 0:NTOK], start=(k == 0), stop=(k == 1))
                    gm = sp.tile([128, NTOK], BF16, tag=f"g{m}", bufs=1)
                    nc.scalar.activation(out=gm, in_=ps, func=AFT.Gelu,
                                         bias=bcol[:, B + 14 + m:B + 15 + m], scale=1.0)
                    gs.append(gm)
                for i in range(2):
                    ps = ps_mm.tile([128, NTOK], F32, tag="m")
                    for k in range(8):
                        mm(ps, wr_l[:, 4096 + 256 * k + 128 * i:4096 + 256 * k + 128 * i + 128],
                           gs[k], start=(k == 0), stop=(k == 7))
                    nc.vector.scalar_tensor_tensor(
                        out=t_res[i], in0=ps, scalar=bcol[:, B + 22 + i:B + 23 + i],
                        in1=t_res[i], op0=ALU.add, op1=ALU.add)

                t2cur = t2nxt
                lncur = lnnxt

            # --- final LN + output ---
            yf = layernorm(0, 2, out_dt=F32)
            for k in range(2):
                nc.sync.dma_start(out=y_own.ap()[128 * k:128 * k + 128, :],
                                  in_=yf[k][:, 0:NTOK])

    nc.compile()
    return nc


def kernel(**inputs):
    x_own, t1pack, t2pack, w, tokmap = _host_prep(inputs)
    if "prog" not in _cache:
        _cache["prog"] = _build_program()
    nc = _cache["prog"]
    in_maps = []
    for c in range(N_CORES):
        m = {"x_own": x_own[c], "t1pack": t1pack[c], "t2pack": t2pack[c]}
        for k, v in w.items():
            m[k] = v
        in_maps.append(m)
    res = run_bass_kernel_spmd(nc, in_maps, core_ids=list(range(N_CORES)),
                               trace=bool(os.environ.get("BASS_TRACE_RUN")))
    _cache["last_results"] = res
    out = np.zeros((DIM, H * W), np.float32)
    for c in range(N_CORES):
        yc = res.results[c]["y_own"]
        valid = tokmap[c] >= 0
        out[:, tokmap[c][valid]] = yc[:, valid]
    return out.reshape(1, DIM, H, W)


# revision 31
# speedup vs baseline: 1.4334x; 1.0133x over previous
"""NATTEN-style dilated neighborhood-attention transformer on 8 trn2 cores.

Design (v2):
- Dilation-3 NA factorizes into 9 independent (row-class s, col-class r)
  blocks; in class space each block is a dense 16 x nc map with a k=13, d=1
  neighborhood.  Sharding: core c owns class-block c (c=0..7); block 8's
  queries are split 42/core (2 class-rows each).  All residual-stream ops
  (LN, QKV, proj, MLP) are token-local; only block-8 attention needs an
  AllGather of each rank's 42-token k/vT slice (~64KB/rank) per layer.
- Attention math per block: masked-dense scores S[k, q] (keys on
  partitions) accumulated in PSUM on top of a host-precomputed additive
  table T2 = rpb_rel*M - 30*(1-M) (per layer/head, bf16, streamed once per
  layer in a single DMA) via an identity-matmul; then one fused
  scalar_tensor_tensor multiply by T1 = ker*M + (1-M) (bf16, resident all
  layers), one Exp per head over the j-merged [128, 3*394] tile, softmax
  denominators via selector-column matmuls accumulated into a [4, NTOK]
  PSUM tile, AV with token-major vT, normalize via a block-broadcast
  matmul + one multiply per 4-head group.
- All weights bf16, packed host-side into one DRAM tensor, loaded once
  into SBUF (resident for all 6 layers); biases/LN params packed; v-bias
  folded into the proj bias; the q-scale folded into qkv weights.  LN uses
  matmul reductions over the f32r residual stream, rstd =
  exp(-0.5*ln(var+eps)) so attention-path ACT ops share one activation
  table set, and broadcast matmuls + fused STT ops for the apply step.
"""
import numpy as np
import sys, os
sys.path.insert(0, "/opt/trn_rl_repo")

import ml_dtypes
import concourse.bass as bass
import concourse.tile as tile
from concourse import mybir, bacc
from concourse.bass_utils import run_bass_kernel_spmd

F32 = mybir.dt.float32
F32R = mybir.dt.float32r
BF16 = mybir.dt.bfloat16
AFT = mybir.ActivationFunctionType
ALU = mybir.AluOpType

KSZ, DIL, SIGMA, SC, EPS = 13, 3, 9.0, 0.1, 1e-5
DEPTH, DIM, HEADS = 6, 256, 8
HD = DIM // HEADS
H, W = 48, 64
NU = H // DIL                      # 16 class rows
NCOLS = [22, 21, 21]               # class cols for r=0,1,2
NB = KSZ // 2                      # 6
NTOK = 394                         # 352 (block, padded) + 42 (block-8 slice)
NQA, NQB = 352, 42
NKEY = 384                         # 3 k-tiles of 128
N_CORES = 8
NEG = -30.0
WCOLS = 6144                       # packed weight cols per layer (bf16)
BCOLS = 24                         # packed bias cols per layer (f32)
NROW = (4 * DEPTH + 2) * 128       # lnrow cols

_cache = {}


def _win_start(L):
    return np.clip(np.arange(L) - NB, 0, L - KSZ)


def _host_prep(inputs):
    """Precompute per-core input tensors (numpy)."""
    x = np.asarray(inputs["x"], np.float32).reshape(DIM, H * W)
    rpb = np.asarray(inputs["rpb"], np.float32)
    lr_m = np.asarray(inputs["lr_m"], np.float32)
    crd = np.arange(KSZ, dtype=np.float32)
    g = np.exp(-((crd[None, :] - KSZ // 2) ** 2 + (crd[:, None] - KSZ // 2) ** 2)
               / (2.0 * SIGMA ** 2))
    kers = [g + lr_m[l] / SC for l in range(DEPTH)]          # (13,13) per layer

    # block token coords: block b=3s+r -> flat hw indices, (u,qc) row-major
    blk_tok = []
    for s in range(3):
        for r in range(3):
            nc_ = NCOLS[r]
            u, qc = np.meshgrid(np.arange(NU), np.arange(nc_), indexing="ij")
            blk_tok.append(((3 * u + s) * W + (3 * qc + r)).reshape(-1))

    # per-block T1/T2 in class space, [nkeys=16*nc, nq=16*nc]
    def block_tables(nc_):
        key = ("bt", nc_)
        if key not in _cache:
            su, sw = _win_start(NU), _win_start(nc_)
            KU, KC, U, QC = np.meshgrid(np.arange(NU), np.arange(nc_),
                                        np.arange(NU), np.arange(nc_), indexing="ij")
            m = ((KU >= su[U]) & (KU <= su[U] + KSZ - 1)
                 & (KC >= sw[QC]) & (KC <= sw[QC] + KSZ - 1))
            i = np.where(m, KU - su[U], 0)
            j = np.where(m, KC - sw[QC], 0)
            rr = np.where(m, KU - U + KSZ - 1, 0)
            cc = np.where(m, KC - QC + KSZ - 1, 0)
            _cache[key] = (m.reshape(NU * nc_, NU * nc_), i.reshape(NU * nc_, -1),
                           j.reshape(NU * nc_, -1), rr.reshape(NU * nc_, -1),
                           cc.reshape(NU * nc_, -1))
        return _cache[key]

    T1 = np.zeros((N_CORES, DEPTH, NKEY, NTOK), np.float32)
    T2 = np.zeros((N_CORES, DEPTH, HEADS, NKEY, NTOK), np.float32)
    x_own = np.zeros((N_CORES, DIM, NTOK), np.float32)
    tokmap = []   # per core: global flat-hw index per col (or -1 pad)
    for c in range(N_CORES):
        ncA = NCOLS[c % 3]
        ntA = NU * ncA
        mA, iA, jA, rA, cA = block_tables(ncA)
        mB, iB, jB, rB, cB = block_tables(21)
        tm = np.full(NTOK, -1, np.int64)
        tm[:ntA] = blk_tok[c]
        sl = slice(42 * c, 42 * c + 42)
        tm[NQA:] = blk_tok[8][sl]
        tokmap.append(tm)
        x_own[c][:, :ntA] = x[:, blk_tok[c]]
        x_own[c][:, NQA:] = x[:, blk_tok[8][sl]]
        for l in range(DEPTH):
            ker = kers[l]
            # piece A: keys rows [0:ntA], queries cols [0:ntA]
            t1A = ker[iA, jA] * mA + (1.0 - mA)
            T1[c, l, :ntA, :ntA] = t1A
            T1[c, l, ntA:, :] = 1.0
            T1[c, l, :, ntA:NQA] = 1.0
            # piece B: keys rows [0:336] (block-8 packed), q cols [NQA:]
            t1B = ker[iB, jB] * mB + (1.0 - mB)
            T1[c, l, :336, NQA:] = t1B[:, sl]
            T1[c, l, 336:, NQA:] = 1.0
            for h in range(HEADS):
                rp = rpb[l, h]
                t2A = rp[rA, cA] * mA + NEG * (1.0 - mA)
                T2[c, l, h, :ntA, :ntA] = t2A
                T2[c, l, h, ntA:, :ntA] = NEG
                T2[c, l, h, :, ntA:NQA] = 0.0
                t2B = rp[rB, cB] * mB + NEG * (1.0 - mB)
                T2[c, l, h, :336, NQA:] = t2B[:, sl]
                T2[c, l, h, 336:, NQA:] = NEG

    BF = ml_dtypes.bfloat16
    # t1pack [C, DEPTH, 128, 3, NTOK]: key = 128*j + p
    t1pack = np.ascontiguousarray(
        T1.reshape(N_CORES, DEPTH, 3, 128, NTOK).transpose(0, 1, 3, 2, 4)
    ).astype(BF)
    # t2pack [C, DEPTH, 128, HEADS, 3, NTOK]
    t2pack = np.ascontiguousarray(
        T2.reshape(N_CORES, DEPTH, HEADS, 3, 128, NTOK).transpose(0, 1, 4, 2, 3, 5)
    ).astype(BF)

    qkv_w = np.asarray(inputs["qkv_w"], np.float32).copy()
    qkv_b = np.asarray(inputs["qkv_b"], np.float32).copy()
    qkv_w[:, :256, :] *= HD ** -0.5          # fold q scale
    qkv_b[:, :256] *= HD ** -0.5
    proj_w = np.asarray(inputs["proj_w"], np.float32)
    proj_b = (np.asarray(inputs["proj_b"], np.float32)
              + np.einsum("lij,lj->li", proj_w, qkv_b[:, 512:]))  # fold v bias
    fc1_w = np.asarray(inputs["fc1_w"], np.float32)
    fc2_w = np.asarray(inputs["fc2_w"], np.float32)

    # wpack [128, DEPTH, WCOLS] bf16; per-layer block:
    # qkw k0(512) k1(512) | vw k0(256) k1(256) | pw k0(256) k1(256)
    # | f1w k0(1024) k1(1024) | f2w 0..7 (2048)
    wpack = np.zeros((128, DEPTH, WCOLS), np.float32)
    for l in range(DEPTH):
        qk_wT = qkv_w[l, :512, :].T          # [256, 512]
        v_wT = qkv_w[l, 512:, :].T           # [256, 256]
        p_wT = proj_w[l].T                   # [256, 256]
        f1_wT = fc1_w[l].T                   # [256, 1024]
        f2_wT = fc2_w[l].T                   # [1024, 256]
        for k in range(2):
            wpack[:, l, 512 * k:512 * k + 512] = qk_wT[128 * k:128 * k + 128]
            wpack[:, l, 1024 + 256 * k:1024 + 256 * k + 256] = v_wT[128 * k:128 * k + 128]
            wpack[:, l, 1536 + 256 * k:1536 + 256 * k + 256] = p_wT[128 * k:128 * k + 128]
            wpack[:, l, 2048 + 1024 * k:2048 + 1024 * k + 1024] = f1_wT[128 * k:128 * k + 128]
        for k in range(8):
            wpack[:, l, 4096 + 256 * k:4096 + 256 * k + 256] = f2_wT[128 * k:128 * k + 128]

    # bcols [128, DEPTH*24 + 4] f32; per layer:
    # ln1_w(2) ln1_b(2) ln2_w(2) ln2_b(2) qk_b(4) proj_b'(2) fc1_b(8) fc2_b(2)
    bcols = np.zeros((128, DEPTH * BCOLS + 4), np.float32)
    for l in range(DEPTH):
        B = BCOLS * l
        for k in range(2):
            bcols[:, B + k] = np.asarray(inputs["ln1_w"], np.float32)[l, 128 * k:128 * k + 128]
            bcols[:, B + 2 + k] = np.asarray(inputs["ln1_b"], np.float32)[l, 128 * k:128 * k + 128]
            bcols[:, B + 4 + k] = np.asarray(inputs["ln2_w"], np.float32)[l, 128 * k:128 * k + 128]
            bcols[:, B + 6 + k] = np.asarray(inputs["ln2_b"], np.float32)[l, 128 * k:128 * k + 128]
        for m in range(4):
            bcols[:, B + 8 + m] = qkv_b[l, 128 * m:128 * m + 128]
        for i in range(2):
            bcols[:, B + 12 + i] = proj_b[l, 128 * i:128 * i + 128]
        for m in range(8):
            bcols[:, B + 14 + m] = np.asarray(inputs["fc1_b"], np.float32)[l, 128 * m:128 * m + 128]
        for i in range(2):
            bcols[:, B + 22 + i] = np.asarray(inputs["fc2_b"], np.float32)[l, 128 * i:128 * i + 128]
    for k in range(2):
        bcols[:, DEPTH * BCOLS + k] = np.asarray(inputs["normf_w"], np.float32)[128 * k:128 * k + 128]
        bcols[:, DEPTH * BCOLS + 2 + k] = np.asarray(inputs["normf_b"], np.float32)[128 * k:128 * k + 128]

    # lnrow [(DEPTH+1), 1024] bf16: per-layer row blocks for the folded
    # broadcast lhsT: w rows at 256*which+128*k, -b rows at 512+256*which+128*k.
    # Row DEPTH holds normf (which=0 slots).
    lnrow = np.zeros((DEPTH + 1, 1024), np.float32)
    for l in range(DEPTH):
        for k in range(2):
            lnrow[l, 128 * k:128 * k + 128] = np.asarray(
                inputs["ln1_w"], np.float32)[l, 128 * k:128 * k + 128]
            lnrow[l, 512 + 128 * k:512 + 128 * k + 128] = -np.asarray(
                inputs["ln1_b"], np.float32)[l, 128 * k:128 * k + 128]
            lnrow[l, 256 + 128 * k:256 + 128 * k + 128] = np.asarray(
                inputs["ln2_w"], np.float32)[l, 128 * k:128 * k + 128]
            lnrow[l, 768 + 128 * k:768 + 128 * k + 128] = -np.asarray(
                inputs["ln2_b"], np.float32)[l, 128 * k:128 * k + 128]
    for k in range(2):
        lnrow[DEPTH, 128 * k:128 * k + 128] = np.asarray(
            inputs["normf_w"], np.float32)[128 * k:128 * k + 128]
        lnrow[DEPTH, 512 + 128 * k:512 + 128 * k + 128] = -np.asarray(
            inputs["normf_b"], np.float32)[128 * k:128 * k + 128]

    # cpack [128, 144] bf16: identity(128) + sel4(16)
    cpack = np.zeros((128, 144), np.float32)
    cpack[:, :128] = np.eye(128, dtype=np.float32)
    for hh in range(4):
        cpack[:, 128 + 4 * hh + hh] = 1.0
    blk4 = np.zeros((4, 128), np.float32)
    for gidx in range(4):
        blk4[gidx, 32 * gidx:32 * gidx + 32] = 1.0

    w = {
        "wpack": wpack.astype(BF), "bcols": bcols,
        "lnrow": lnrow.astype(BF),
        "cpack": cpack.astype(BF), "blk4": blk4,
        "rowc": np.ones((1, 128), np.float32),
        "colc": np.full((128, 1), 1.0 / DIM, np.float32),
    }
    return x_own, t1pack, t2pack, w, tokmap


# ag_in layout per rank (flat elems): qk-slice [512,42] then vT-slice [42,256]
AG_QK, AG_VT = 512 * 42, 42 * 256
AG_N = AG_QK + AG_VT


def _build_program(skip_ag=False):
    nc = bacc.Bacc("TRN2", target_bir_lowering=False, debug=False,
                   num_devices=1 if skip_ag else N_CORES)
    dram = {}
    def din(name, shape, dt=F32):
        dram[name] = nc.dram_tensor(name, list(shape), dt, kind="ExternalInput")
        return dram[name]

    din("x_own", (DIM, NTOK), F32R)
    din("t1pack", (DEPTH, 128, 3, NTOK), BF16)
    din("t2pack", (DEPTH, 128, HEADS, 3, NTOK), BF16)
    din("wpack", (128, DEPTH, WCOLS), BF16)
    din("bcols", (128, DEPTH * BCOLS + 4))
    din("lnrow", (DEPTH + 1, 1024), BF16)
    din("cpack", (128, 144), BF16)
    din("blk4", (4, 128), F32R)
    din("rowc", (1, 128), F32R)
    din("colc", (128, 1), F32R)
    y_own = nc.dram_tensor("y_own", [DIM, NTOK], F32, kind="ExternalOutput")
    ag_in = [nc.dram_tensor(f"ag_in{l}", [AG_N], BF16) for l in range(DEPTH)]
    ag_out = [nc.dram_tensor(f"ag_out{l}", [N_CORES * AG_N], BF16,
                             addr_space="Shared") for l in range(DEPTH)]

    def mm(out, lhsT, rhs, start, stop, tp=None, skip=False):
        kw = {}
        if tp is not None:
            kw["tile_position"] = tp
        if skip:
            kw["skip_group_check"] = True
        nc.tensor.matmul(out, lhsT, rhs, start=start, stop=stop, **kw)

    with tile.TileContext(nc) as tc:
        import contextlib
        with contextlib.ExitStack() as ctx:
            ctx.enter_context(nc.allow_low_precision(
                reason="bf16 weights/activations; f32r residual stream"))
            cn = ctx.enter_context(tc.tile_pool(name="cn", bufs=1))
            t2p = ctx.enter_context(tc.tile_pool(name="t2p", bufs=2))
            sp = ctx.enter_context(tc.tile_pool(name="sp", bufs=2))
            x2p = ctx.enter_context(tc.tile_pool(name="x2p", bufs=3))
            pjp = ctx.enter_context(tc.tile_pool(name="pjp", bufs=3))
            ps_s = ctx.enter_context(tc.tile_pool(name="ps_s", bufs=3, space="PSUM"))
            ps_at = ctx.enter_context(tc.tile_pool(name="ps_at", bufs=2, space="PSUM"))
            ps_sm = ctx.enter_context(tc.tile_pool(name="ps_sm", bufs=1, space="PSUM"))
            ps_mm = ctx.enter_context(tc.tile_pool(name="ps_mm", bufs=2, space="PSUM"))

            # ---- persistent tiles / constants ----
            wres = [cn.tile([128, WCOLS], BF16, name=f"wres{l}") for l in range(DEPTH)]
            t1res = [cn.tile([128, 3 * NTOK], BF16, name=f"t1res{l}") for l in range(DEPTH)]
            bcol = cn.tile([128, DEPTH * BCOLS + 4], F32, name="bcol")

            cpk = cn.tile([128, 144], BF16, name="cpk")
            blk4 = cn.tile([4, 128], F32R, name="blk4")
            ones_row = cn.tile([1, 128], F32R, name="ones_row")
            ones_sc = cn.tile([128, 1], F32R, name="ones_sc")
            eps_t = cn.tile([1, 1], F32, name="eps_t")
            nc.sync.dma_start(out=ones_row, in_=dram["rowc"].ap())
            nc.sync.dma_start(out=ones_sc, in_=dram["colc"].ap())
            nc.vector.memset(eps_t, EPS)

            # residual stream, feature-major [2][128, NTOK]
            t_res = [cn.tile([128, NTOK], F32R, name=f"t_res{i}") for i in range(2)]
            for i in range(2):
                nc.sync.dma_start(out=t_res[i],
                                  in_=dram["x_own"].ap()[128 * i:128 * (i + 1), :])
            nc.sync.dma_start(out=bcol, in_=dram["bcols"].ap())
            nc.sync.dma_start(out=cpk, in_=dram["cpack"].ap())
            nc.sync.dma_start(out=blk4, in_=dram["blk4"].ap())
            ident = cpk[:, 0:128]

            def wres_load(l):
                nc.sync.dma_start(
                    out=wres[l],
                    in_=bass.AP(tensor=dram["wpack"], offset=WCOLS * l,
                                ap=[[DEPTH * WCOLS, 128], [1, WCOLS]]))

            def t1_load(l):
                nc.sync.dma_start(
                    out=t1res[l],
                    in_=bass.AP(tensor=dram["t1pack"],
                                offset=l * 128 * 3 * NTOK,
                                ap=[[3 * NTOK, 128], [1, 3 * NTOK]]))

            def ln_load(l):
                t = sp.tile([1, 1024], BF16, tag="lnr", bufs=2, name=f"lnr_{l}")
                nc.sync.dma_start(out=t, in_=dram["lnrow"].ap()[l:l + 1, :])
                return t

            def t2_load(l):
                t = t2p.tile([128, HEADS * 3 * NTOK], BF16, tag="t2", name=f"t2_{l}")
                nc.sync.dma_start(
                    out=t,
                    in_=bass.AP(tensor=dram["t2pack"],
                                offset=l * 128 * HEADS * 3 * NTOK,
                                ap=[[HEADS * 3 * NTOK, 128], [1, HEADS * 3 * NTOK]]))
                return t

            wres_load(0)
            t1_load(0)
            lncur = ln_load(0)
            t2cur = t2_load(0)

            def layernorm(lidx, which, out_dt=BF16, out_w=None):
                """which: 0=ln1, 1=ln2, 2=normf. Returns 2 feature-half tiles.
                y_k = (t_k * w_k - (w_k (x) mean - b_k (x) sd)) * (1 (x) rstd)
                """
                lnr = lncur
                if which == 2:
                    wc = [bcol[:, DEPTH * BCOLS + k:DEPTH * BCOLS + k + 1] for k in range(2)]
                    wsel = 0
                else:
                    B = BCOLS * lidx
                    wc = [bcol[:, B + 4 * which + k:B + 4 * which + k + 1] for k in range(2)]
                    wsel = which
                wr = [(lnr[0:1, 256 * wsel + 128 * k:256 * wsel + 128 * k + 128],
                       lnr[0:1, 512 + 256 * wsel + 128 * k:512 + 256 * wsel + 128 * k + 128])
                      for k in range(2)]
                su1 = ps_mm.tile([1, NTOK], F32, tag="m")
                su2 = ps_mm.tile([1, NTOK], F32, tag="m")
                sqs = []
                for k in range(2):
                    sq = sp.tile([128, NTOK], F32R, tag="sq", bufs=2)
                    nc.gpsimd.tensor_mul(sq, t_res[k], t_res[k])
                    sqs.append(sq)
                for k in range(2):
                    mm(su1, ones_sc, t_res[k], start=(k == 0), stop=(k == 1))
                for k in range(2):
                    mm(su2, ones_sc, sqs[k], start=(k == 0), stop=(k == 1))
                mean_sb = sp.tile([1, NTOK], BF16, tag="lns", bufs=5)
                nc.vector.tensor_copy(mean_sb, su1)
                m2 = sp.tile([1, NTOK], F32R, tag="lns", bufs=5)
                nc.vector.scalar_tensor_tensor(out=m2, in0=su1, scalar=1.0,
                                               in1=mean_sb, op0=ALU.bypass,
                                               op1=ALU.mult)
                var = sp.tile([1, NTOK], F32, tag="lns", bufs=5)
                nc.vector.scalar_tensor_tensor(out=var, in0=su2, scalar=1.0,
                                               in1=m2, op0=ALU.bypass, op1=ALU.subtract)
                sd = sp.tile([1, NTOK], BF16, tag="lns", bufs=5)
                nc.scalar.activation(out=sd, in_=var, func=AFT.Sqrt,
                                     bias=eps_t, scale=1.0)
                rstd = sp.tile([1, NTOK], F32R, tag="lns", bufs=5)
                nc.vector.reciprocal(rstd, sd)
                bc_r = sp.tile([128, NTOK], F32R, tag="bcr", bufs=2)
                nc.gpsimd.partition_broadcast(bc_r, rstd, channels=128)
                y = []
                for k in range(2):
                    bc_f = ps_mm.tile([128, NTOK], F32, tag="m")
                    mm(bc_f, wr[k][0], mean_sb, start=True, stop=False)
                    mm(bc_f, wr[k][1], sd, start=False, stop=True)
                    u = sp.tile([128, NTOK], BF16, tag="ln_u", bufs=2)
                    nc.vector.scalar_tensor_tensor(out=u, in0=t_res[k], scalar=wc[k],
                                                   in1=bc_f, op0=ALU.mult, op1=ALU.subtract)
                    wy = out_w if out_w is not None else NTOK
                    yk = sp.tile([128, wy], out_dt, tag=f"y{which}{k}",
                                 bufs=1 if which == 2 else 2)
                    nc.gpsimd.tensor_mul(yk[:, 0:NTOK], u, bc_r)
                    y.append(yk)
                return y

            def preload(func, anchor):
                junk = sp.tile([1, 1], F32, tag="junk", bufs=1)
                nc.scalar.activation(out=junk, in_=anchor, func=func)

            for l in range(DEPTH):
                B = BCOLS * l
                wr_l = wres[l]

                # --- LN1 ---  (y1 tiles 512 wide; tail cols junk, outputs unread)
                y1 = layernorm(l, 0, out_dt=BF16, out_w=512)

                # --- qk projection (feature-major, single merged tile) ---
                qk_sb = sp.tile([128, 4 * NTOK], BF16, tag="qk", bufs=1)
                for m in range(4):
                    pool = ps_mm if m % 2 == 0 else ps_at
                    ps = pool.tile([128, NTOK], F32, tag="m" if m % 2 == 0 else "at")
                    for k in range(2):
                        mm(ps, wr_l[:, 512 * k + 128 * m:512 * k + 128 * m + 128],
                           y1[k][:, 0:NTOK], start=(k == 0), stop=(k == 1))
                    nc.scalar.activation(out=qk_sb[:, NTOK * m:NTOK * (m + 1)],
                                         in_=ps, func=AFT.Identity,
                                         bias=bcol[:, B + 8 + m:B + 9 + m], scale=1.0)

                # --- vT (token-major, single merged tile [128, 4*256]) ---
                vtt = sp.tile([128, 4 * 256], BF16, tag="vt", bufs=1)
                for mt in range(4):
                    npt = min(128, NTOK - 128 * mt)
                    pool = ps_mm if mt % 2 == 0 else ps_at
                    ps = pool.tile([128, 256], F32, tag="m" if mt % 2 == 0 else "at")
                    for k in range(2):
                        mm(ps, y1[k][:, 128 * mt:128 * mt + 128],
                           wr_l[:, 1024 + 256 * k:1024 + 256 * k + 256],
                           start=(k == 0), stop=(k == 1))
                    if mt % 2 == 0:
                        nc.vector.tensor_copy(vtt[:npt, 256 * mt:256 * mt + 256],
                                              ps[:npt, :])
                    else:
                        nc.scalar.activation(out=vtt[:npt, 256 * mt:256 * mt + 256],
                                             in_=ps[:npt, :], func=AFT.Copy)

                # --- AG of block-8 slice ---
                nc.sync.dma_start(
                    out=bass.AP(tensor=ag_in[l], offset=0,
                                ap=[[42, 128], [128 * 42, 4], [1, 42]]),
                    in_=bass.AP(tensor=qk_sb.tensor, offset=qk_sb.offset + NQA,
                                ap=[qk_sb.ap[0], [NTOK, 4], [1, 42]]))
                nc.sync.dma_start(
                    out=bass.AP(tensor=ag_in[l], offset=AG_QK,
                                ap=[[256, 32], [1, 256]]),
                    in_=vtt[96:128, 512:768])
                nc.sync.dma_start(
                    out=bass.AP(tensor=ag_in[l], offset=AG_QK + 32 * 256,
                                ap=[[256, 10], [1, 256]]),
                    in_=vtt[0:10, 768:1024])
                if not skip_ag:
                    nc.gpsimd.collective_compute(
                        "AllGather", ALU.bypass,
                        replica_groups=[list(range(N_CORES))],
                        ins=[ag_in[l].ap().opt()], outs=[ag_out[l].ap().opt()])

                # --- gather block-8 K (feature-major) and vT (token-major) ---
                kb = [sp.tile([128, NKEY], BF16, tag=f"kb{i}", name=f"kb{i}")
                      for i in range(2)]
                for i in range(2):
                    nc.sync.dma_start(
                        out=bass.AP(tensor=kb[i].tensor, offset=kb[i].offset,
                                    ap=[kb[i].ap[0], [42, 8], [1, 42]]),
                        in_=bass.AP(tensor=ag_out[l],
                                    offset=(256 + 128 * i) * 42,
                                    ap=[[42, 128], [AG_N, 8], [1, 42]]))
                vb = [sp.tile([128, 256], BF16, tag=f"vb{i}", name=f"vb{i}")
                      for i in range(3)]
                for r in range(8):
                    row0 = 42 * r
                    left = 42
                    while left > 0:
                        ti, ri = row0 // 128, row0 % 128
                        cnt = min(left, 128 - ri)
                        nc.sync.dma_start(
                            out=vb[ti][ri:ri + cnt, :],
                            in_=bass.AP(tensor=ag_out[l],
                                        offset=AG_N * r + AG_QK + (row0 - 42 * r) * 256,
                                        ap=[[256, cnt], [1, 256]]))
                        row0 += cnt; left -= cnt

                # prefetch next layer's tables/weights (behind the gathers
                # in the DMA FIFO so they don't stall this layer)
                if l + 1 < DEPTH:
                    t2nxt = t2_load(l + 1)
                    wres_load(l + 1)
                    t1_load(l + 1)
                    lnnxt = ln_load(l + 1)
                else:
                    t2nxt = None
                    lnnxt = ln_load(DEPTH)

                # --- attention ---
                attn_sb = [sp.tile([128, NTOK], BF16, tag=f"at{g}", name=f"attn_sb{g}")
                           for g in range(2)]
                for g in range(2):
                    at_ps = ps_at.tile([128, NTOK], F32, tag="at")
                    sm = ps_sm.tile([4, NTOK], F32, tag="sm")

                    def emit_front(hh):
                        h = 4 * g + hh
                        hr = slice(32 * hh, 32 * hh + 32)
                        tp = (96, 0) if hh == 3 else None
                        x2 = x2p.tile([128, 3 * NTOK], F32, tag="x2", name="x2")
                        for j in range(3):
                            s_ps = ps_s.tile([128, NTOK], F32, tag="s")
                            # T2 first (start), then S_A, S_B accumulate
                            mm(s_ps, ident,
                               t2cur[:, (h * 3 + j) * NTOK:(h * 3 + j + 1) * NTOK],
                               start=True, stop=False, skip=True)
                            mm(s_ps[:, 0:NQA],
                               qk_sb[hr, NTOK * (2 + g) + 128 * j:NTOK * (2 + g) + 128 * j + 128],
                               qk_sb[hr, NTOK * g:NTOK * g + NQA],
                               start=False, stop=False, tp=tp, skip=True)
                            mm(s_ps[:, NQA:NTOK],
                               kb[g][hr, 128 * j:128 * j + 128],
                               qk_sb[hr, NTOK * g + NQA:NTOK * (g + 1)],
                               start=False, stop=True, tp=tp, skip=True)
                            # x2 = (S + T2) * T1   (DVE: only engine with PSUM reads)
                            nc.vector.scalar_tensor_tensor(
                                out=x2[:, NTOK * j:NTOK * (j + 1)], in0=s_ps,
                                scalar=1.0,
                                in1=t1res[l][:, j * NTOK:(j + 1) * NTOK],
                                op0=ALU.bypass, op1=ALU.mult)
                        pj = pjp.tile([128, 3 * NTOK], BF16, tag="pj", name="pj")
                        nc.scalar.activation(out=pj, in_=x2, func=AFT.Exp)
                        return pj

                    def emit_back(hh, pj):
                        h = 4 * g + hh
                        hr = slice(32 * hh, 32 * hh + 32)
                        # denominators: sel-column matmuls into [4, NTOK]
                        for j in range(3):
                            mm(sm, cpk[:, 128 + 4 * hh:128 + 4 * hh + 4],
                               pj[:, NTOK * j:NTOK * (j + 1)],
                               start=(hh == 0 and j == 0), stop=(hh == 3 and j == 2),
                               skip=True)
                        # AV
                        tpav = (0, 96) if hh == 3 else None
                        for j in range(3):
                            mm(at_ps[hr, 0:NQA], vtt[:, 256 * j + 32 * h:256 * j + 32 * h + 32],
                               pj[:, NTOK * j:NTOK * j + NQA],
                               start=(j == 0), stop=(j == 2), tp=tpav, skip=True)
                            mm(at_ps[hr, NQA:NTOK], vb[j][:, 32 * h:32 * h + 32],
                               pj[:, NTOK * j + NQA:NTOK * (j + 1)],
                               start=(j == 0), stop=(j == 2), tp=tpav, skip=True)

                    from collections import deque
                    pend = deque()
                    for hh in range(4):
                        pj = emit_front(hh)
                        pend.append((hh, pj))
                        if len(pend) > 2:
                            emit_back(*pend.popleft())
                    while pend:
                        emit_back(*pend.popleft())
                    inv4 = sp.tile([4, NTOK], F32R, tag="inv", bufs=2)
                    nc.vector.reciprocal(inv4, sm)
                    bc = ps_mm.tile([128, NTOK], F32, tag="m")
                    mm(bc, blk4, inv4, start=True, stop=True)
                    bc_sb = sp.tile([128, NTOK], F32R, tag="bc_sb", bufs=2)
                    nc.scalar.activation(out=bc_sb, in_=bc, func=AFT.Copy)
                    nc.vector.scalar_tensor_tensor(out=attn_sb[g], in0=at_ps,
                                                   scalar=1.0, in1=bc_sb,
                                                   op0=ALU.bypass, op1=ALU.mult)

                # --- proj + residual ---
                for i in range(2):
                    ps = ps_mm.tile([128, NTOK], F32, tag="m")
                    for k in range(2):
                        mm(ps, wr_l[:, 1536 + 256 * k + 128 * i:1536 + 256 * k + 128 * i + 128],
                           attn_sb[k], start=(k == 0), stop=(k == 1))
                    nc.vector.scalar_tensor_tensor(
                        out=t_res[i], in0=ps, scalar=bcol[:, B + 12 + i:B + 13 + i],
                        in1=t_res[i], op0=ALU.add, op1=ALU.add)

                # --- LN2 + MLP ---
                y2 = layernorm(l, 1, out_dt=BF16)
                gs = []
                for m in range(8):
                    ps = ps_at.tile([128, NTOK], F32, tag="at")
                    for k in range(2):
                        mm(ps, wr_l[:, 2048 + 1024 * k + 128 * m:2048 + 1024 * k + 128 * m + 128],
                           y2[k][:, 0:NTOK], start=(k == 0), stop=(k == 1))
                    gm = sp.tile([128, NTOK], BF16, tag=f"g{m}", bufs=1)
                    nc.scalar.activation(out=gm, in_=ps, func=AFT.Gelu,
                                         bias=bcol[:, B + 14 + m:B + 15 + m], scale=1.0)
                    gs.append(gm)
                for i in range(2):
                    ps = ps_mm.tile([128, NTOK], F32, tag="m")
                    for k in range(8):
                        mm(ps, wr_l[:, 4096 + 256 * k + 128 * i:4096 + 256 * k + 128 * i + 128],
                           gs[k], start=(k == 0), stop=(k == 7))
                    nc.vector.scalar_tensor_tensor(
                        out=t_res[i], in0=ps, scalar=bcol[:, B + 22 + i:B + 23 + i],
                        in1=t_res[i], op0=ALU.add, op1=ALU.add)

                t2cur = t2nxt
                lncur = lnnxt

            # --- final LN + output ---
            yf = layernorm(0, 2, out_dt=F32)
            for k in range(2):
                nc.sync.dma_start(out=y_own.ap()[128 * k:128 * k + 128, :],
                                  in_=yf[k][:, 0:NTOK])

    nc.compile()
    return nc


def kernel(**inputs):
    x_own, t1pack, t2pack, w, tokmap = _host_prep(inputs)
    if "prog" not in _cache:
        _cache["prog"] = _build_program()
    nc = _cache["prog"]
    in_maps = []
    for c in range(N_CORES):
        m = {"x_own": x_own[c], "t1pack": t1pack[c], "t2pack": t2pack[c]}
        for k, v in w.items():
            m[k] = v
        in_maps.append(m)
    res = run_bass_kernel_spmd(nc, in_maps, core_ids=list(range(N_CORES)),
                               trace=bool(os.environ.get("BASS_TRACE_RUN")))
    _cache["last_results"] = res
    out = np.zeros((DIM, H * W), np.float32)
    for c in range(N_CORES):
        yc = res.results[c]["y_own"]
        valid = tokmap[c] >= 0
        out[:, tokmap[c][valid]] = yc[:, valid]
    return out.reshape(1, DIM, H, W)
